# revision 1
# baseline (speedup 1.0000x reference)
"""Trainium2 Bass kernel for nn_ConditionalDisCoLoss.

loss = BCEWithLogits(inputs, targets)
     + dCor_masked(sigmoid(inputs), spectators, mask=spectators>=0.5)

Reformulation (no centered n x n matrices):
  p = sigmoid(x), m = (s >= 0.5), c = max(sum m, 1)
  A_i = sum_j m_i m_j |p_i - p_j|,  B_i likewise for s
  Sxy = sum_ij m_i m_j |p_i-p_j||s_i-s_j|
  Sxx = 2c*sum(m p^2) - 2(sum m p)^2   (closed form), Syy likewise
  Vxy = Sxy - (2/c) sum A_i B_i + (sum A)(sum B)/c^2  (and Vxx, Vyy)
  dcor = sqrt(max(Vxy,eps'))/sqrt(...)   with the reference's eps placement

Distribution + symmetry: the pair matrix is symmetric, so only j-bands
at or above each row's band are computed.  Global i-tiles (128 rows) are
dealt round-robin: core k owns i-tiles {8*it + k}, whose band is exactly
`it`, so every core runs the SAME program (jt in [it, 8)) on different
gathered row data - 36 of 64 tiles each.

Per tile [128 x 1024]:
 - PE: masked pairwise diffs D1 = m_i m_j (p_i - p_j) via K=4 bf16
   matmuls (bf16 hi+lo split of p keeps ~1e-7 element accuracy)
 - ACT: U = |D1| (bf16) + fused row-sum accum (A row-part); part of |D2|
 - DVE: rest of |D2| via abs_max + fused accum; product U*V with fused
   row-sum accum (Sxy partials)
 - PE: column sums of U,V for strictly-upper tiles (the transposed
   pairs' row sums) via [128,128]^T @ ones accumulated in one PSUM bank
Host combines per-core partial A/B vectors and scalars in float64.
"""

import numpy as np
from contextlib import ExitStack

import concourse.bass as bass
import concourse.bacc as bacc
import concourse.tile as tile
from concourse import mybir
from concourse.bass_utils import run_bass_kernel_spmd

N = 8192
NCORES = 8
STRIP = N // NCORES      # 1024 rows per core (gathered, not contiguous)
P = 128
JT = 1024                # j-tile width (one band = one j-tile)
NB = N // JT             # 8 bands
NIT = STRIP // P         # 8 i-tiles per core; i-tile it sits in band it
F_FULL = N // P          # 64
F_STRIP = STRIP // P     # 8
SPLIT_ACT = 704          # columns of |D2| done on ACT; rest on DVE

F32 = mybir.dt.float32
BF16 = mybir.dt.bfloat16
F32R = mybir.dt.float32r
ALU = mybir.AluOpType
ACTF = mybir.ActivationFunctionType
AX = mybir.AxisListType

NOUT = 16
# partials slots: 5 sum(R_diag), 6 sum(m), 7 sum(m*p), 8 sum(m*p^2),
#                 9 sum(m*s), 10 sum(m*s^2), 11 sum(bce), 12 sum(R_upper)
NCOLP = 112  # colparts: 7 bands x (8 quarters A | 8 quarters B)


def _build():
    nc = bacc.Bacc("TRN2", target_bir_lowering=False, debug=False,
                   num_devices=NCORES, enable_asserts=False)

    x_full = nc.dram_tensor("x_full", [N, 1], F32, kind="ExternalInput")
    s_full = nc.dram_tensor("s_full", [N], F32, kind="ExternalInput")
    x_strip = nc.dram_tensor("x_strip", [STRIP, 1], F32, kind="ExternalInput")
    t_strip = nc.dram_tensor("t_strip", [STRIP, 1], F32, kind="ExternalInput")
    s_strip = nc.dram_tensor("s_strip", [STRIP], F32, kind="ExternalInput")
    out = nc.dram_tensor("partials", [NOUT], F32, kind="ExternalOutput")
    rowp = nc.dram_tensor("rowparts", [P, 16], F32, kind="ExternalOutput")
    colp = nc.dram_tensor("colparts", [P, NCOLP], F32, kind="ExternalOutput")

    with tile.TileContext(nc) as tc, ExitStack() as ctx:
        pre = ctx.enter_context(tc.tile_pool(name="pre", bufs=1))
        uvp = ctx.enter_context(tc.tile_pool(name="uv", bufs=3))
        accp = ctx.enter_context(tc.tile_pool(name="acc", bufs=2))
        psp = ctx.enter_context(tc.tile_pool(name="psp", bufs=3, space="PSUM"))
        psc = ctx.enter_context(tc.tile_pool(name="psc", bufs=1, space="PSUM"))

        # ---------- preprocessing: full vectors -> moving operands ----------
        xf = pre.tile([P, F_FULL], F32)
        sf = pre.tile([P, F_FULL], F32)
        nc.sync.dma_start(out=xf, in_=x_full.ap().rearrange("(p f) one -> p (f one)", p=P))
        nc.scalar.dma_start(out=sf, in_=s_full.ap().rearrange("(p f) -> p f", p=P))

        pf = pre.tile([P, F_FULL], F32)
        nc.scalar.activation(pf, xf, ACTF.Sigmoid)
        mf = pre.tile([P, F_FULL], F32)
        nc.vector.tensor_scalar(mf, sf, 0.5, None, ALU.is_ge)
        af = pre.tile([P, F_FULL], F32)
        nc.vector.tensor_tensor(out=af, in0=mf, in1=pf, op=ALU.mult)
        cf = pre.tile([P, F_FULL], F32)
        nc.vector.tensor_tensor(out=cf, in0=mf, in1=sf, op=ALU.mult)

        # moving operands (f32, fed to the PE as float32r via bitcast):
        # RA rows: m, a=m*p   RB rows: m, c=m*s
        RA = pre.tile([2, N], F32)
        RB = pre.tile([2, N], F32)
        for eng, dst, row, src in ((nc.sync, RA, 0, mf), (nc.scalar, RA, 1, af),
                                   (nc.sync, RB, 0, mf), (nc.scalar, RB, 1, cf)):
            eng.dma_start(out=dst[row:row + 1, :], in_=src)

        # ---------- preprocessing: gathered strip -> stationary operands ----------
        # [16, 64] layout: strip position s = p*64 + f (DMA-friendly 256B rows)
        PS, FS = 16, 64
        xs = pre.tile([PS, FS], F32)
        ts = pre.tile([PS, FS], F32)
        ss = pre.tile([PS, FS], F32)
        nc.sync.dma_start(out=xs, in_=x_strip.ap().rearrange("(p f) one -> p (f one)", p=PS))
        nc.scalar.dma_start(out=ts, in_=t_strip.ap().rearrange("(p f) one -> p (f one)", p=PS))
        nc.sync.dma_start(out=ss, in_=s_strip.ap().rearrange("(p f) -> p f", p=PS))

        ps_ = pre.tile([PS, FS], F32)
        nc.scalar.activation(ps_, xs, ACTF.Sigmoid)
        ms = pre.tile([PS, FS], F32)
        nc.vector.tensor_scalar(ms, ss, 0.5, None, ALU.is_ge)
        negm = pre.tile([PS, FS], F32)
        nc.vector.tensor_scalar(negm, ms, -1.0, None, ALU.mult)

        bs = pre.tile([PS, FS], F32)
        nc.vector.tensor_tensor(out=bs, in0=ms, in1=ps_, op=ALU.mult)
        ds = pre.tile([PS, FS], F32)
        nc.vector.tensor_tensor(out=ds, in0=ms, in1=ss, op=ALU.mult)

        # stationary operands: LA rows (b, -m), LB rows (d, -m)
        LA = pre.tile([2, STRIP], F32)
        LB = pre.tile([2, STRIP], F32)
        for eng, dst, row, src in ((nc.sync, LA, 0, bs), (nc.scalar, LA, 1, negm),
                                   (nc.sync, LB, 0, ds), (nc.scalar, LB, 1, negm)):
            eng.dma_start(out=dst[row:row + 1, :], in_=src)

        # ---------- O(n) scalar columns (strip tiles live on partitions 0:16,
        # rest of cat stays zero and drops out of the final ones-matmul) ----------
        cat = pre.tile([P, NOUT], F32)
        nc.vector.memset(cat, 0.0)
        junk_s = pre.tile([PS, FS], F32)

        nc.vector.tensor_reduce(cat[0:PS, 6:7], ms, AX.X, ALU.add)
        nc.vector.tensor_reduce(cat[0:PS, 7:8], bs, AX.X, ALU.add)
        nc.vector.scalar_tensor_tensor(out=junk_s, in0=bs, scalar=0.0,
                                       in1=ps_, op0=ALU.bypass, op1=ALU.mult,
                                       accum_out=cat[0:PS, 8:9])
        nc.vector.tensor_reduce(cat[0:PS, 9:10], ds, AX.X, ALU.add)
        junk_s2 = pre.tile([PS, FS], F32)
        nc.vector.scalar_tensor_tensor(out=junk_s2, in0=ds, scalar=0.0,
                                       in1=ss, op0=ALU.bypass, op1=ALU.mult,
                                       accum_out=cat[0:PS, 10:11])

        # ---------- main pass: tiles (it, jt) with jt >= it ----------
        ones = pre.tile([P, 1], BF16)
        nc.vector.memset(ones, 1.0)
        onesf = pre.tile([P, 1], F32)
        nc.vector.memset(onesf, 1.0)

        # per-tile column sums, rectangular [it][jt][16] layout (no PSUM
        # accumulation -- scheduler may reorder same-engine matmuls, so
        # every tile writes its own fresh column; reduced over it at the end)
        colacc = psc.tile([P, NIT, NB, 16], F32)

        AA = pre.tile([P, NIT], F32)
        BB = pre.tile([P, NIT], F32)
        RRd = pre.tile([P, NIT], F32)
        RRu = pre.tile([P, NIT], F32)
        nc.vector.memset(RRu, 0.0)

        for it in range(NIT):
            njt = NB - it
            Ap = accp.tile([P, NB], F32, tag="Ap")
            Bp = accp.tile([P, 2 * NB], F32, tag="Bp")
            Rp = accp.tile([P, NB], F32, tag="Rp")
            lA = LA[:, it * P:(it + 1) * P]
            lB = LB[:, it * P:(it + 1) * P]
            for jj in range(njt):
                jt = it + jj
                psA = psp.tile([P, JT], F32, tag="ps")
                psB = psp.tile([P, JT], F32, tag="ps")
                for h in range(JT // 512):
                    j0 = jt * JT + h * 512
                    nc.tensor.matmul(psA[:, h * 512:(h + 1) * 512],
                                     lhsT=lA.bitcast(F32R),
                                     rhs=RA[:, j0:j0 + 512].bitcast(F32R),
                                     start=True, stop=True)
                    nc.tensor.matmul(psB[:, h * 512:(h + 1) * 512],
                                     lhsT=lB.bitcast(F32R),
                                     rhs=RB[:, j0:j0 + 512].bitcast(F32R),
                                     start=True, stop=True)
                U = uvp.tile([P, JT], BF16, tag="U")
                V = uvp.tile([P, JT], F32, tag="V")
                nc.scalar.activation(U, psA, ACTF.Abs, accum_out=Ap[:, jj:jj + 1])
                nc.scalar.activation(V[:, 0:SPLIT_ACT], psB[:, 0:SPLIT_ACT], ACTF.Abs,
                                     accum_out=Bp[:, 2 * jj:2 * jj + 1])
                # |x| on DVE in 2 ops (only one PSUM operand allowed per op):
                # Vn = -psB_slice (PSUM->SBUF), then V2 = max(Vn, psB_slice)
                Vn = uvp.tile([P, JT - SPLIT_ACT], F32, tag="Vn")
                nc.vector.tensor_scalar(Vn, psB[:, SPLIT_ACT:JT], -1.0, None, ALU.mult)
                nc.vector.scalar_tensor_tensor(out=V[:, SPLIT_ACT:JT],
                                               in0=Vn, scalar=0.0,
                                               in1=psB[:, SPLIT_ACT:JT],
                                               op0=ALU.bypass, op1=ALU.max,
                                               accum_out=Bp[:, 2 * jj + 1:2 * jj + 2])
                W = uvp.tile([P, JT], F32, tag="W")
                nc.vector.scalar_tensor_tensor(out=W, in0=U, scalar=0.0,
                                               in1=V, op0=ALU.bypass, op1=ALU.mult,
                                               accum_out=Rp[:, jj:jj + 1])
                if jt > it:
                    # transposed pairs' row sums = column sums, via PE
                    for q in range(8):
                        nc.tensor.matmul(colacc[:, it, jt, q:q + 1],
                                         lhsT=U[:, q * P:(q + 1) * P], rhs=ones,
                                         start=True, stop=True)
                        nc.tensor.matmul(colacc[:, it, jt, q + 8:q + 9],
                                         lhsT=V[:, q * P:(q + 1) * P], rhs=onesf,
                                         start=True, stop=True)
            nc.vector.tensor_reduce(AA[:, it:it + 1], Ap[:, 0:njt], AX.X, ALU.add)
            nc.vector.tensor_reduce(BB[:, it:it + 1], Bp[:, 0:2 * njt], AX.X, ALU.add)
            nc.vector.tensor_copy(RRd[:, it:it + 1], Rp[:, 0:1])
            if njt > 1:
                nc.vector.tensor_reduce(RRu[:, it:it + 1], Rp[:, 1:njt], AX.X, ALU.add)

        # ---------- outputs ----------
        # BCE partial: relu(x) - x*t + softplus(-|x|) = relu - xt + ln(1+exp(-|x|))
        rx = pre.tile([PS, FS], F32)
        nc.vector.tensor_scalar(rx, xs, 0.0, None, ALU.max)
        xt = pre.tile([PS, FS], F32)
        nc.vector.tensor_tensor(out=xt, in0=xs, in1=ts, op=ALU.mult)
        axx = pre.tile([PS, FS], F32)
        nc.scalar.activation(axx, xs, ACTF.Abs)
        enx = pre.tile([PS, FS], F32)
        nc.scalar.activation(enx, axx, ACTF.Exp, scale=-1.0)
        sp = pre.tile([PS, FS], F32)
        nc.scalar.activation(sp, enx, ACTF.Ln, bias=1.0)
        t1 = pre.tile([PS, FS], F32)
        nc.vector.tensor_tensor(out=t1, in0=rx, in1=xt, op=ALU.subtract)
        t2 = pre.tile([PS, FS], F32)
        nc.vector.scalar_tensor_tensor(out=t2, in0=t1, scalar=0.0, in1=sp,
                                       op0=ALU.add, op1=ALU.add,
                                       accum_out=cat[0:PS, 11:12])

        nc.vector.tensor_reduce(cat[:, 5:6], RRd, AX.X, ALU.add)
        nc.vector.tensor_reduce(cat[:, 12:13], RRu, AX.X, ALU.add)

        pcat = psp.tile([NOUT, 1], F32, tag="ps")
        nc.tensor.matmul(pcat, lhsT=cat, rhs=onesf, start=True, stop=True)
        outt = pre.tile([NOUT, 1], F32)
        nc.scalar.copy(outt, pcat)
        nc.sync.dma_start(out=out.ap().rearrange("(a b) -> a b", b=1), in_=outt)

        rowt = pre.tile([P, 16], F32)
        nc.vector.tensor_copy(rowt[:, 0:8], AA)
        nc.vector.tensor_copy(rowt[:, 8:16], BB)
        nc.sync.dma_start(out=rowp.ap(), in_=rowt)

        # reduce per-tile column sums over it (strided AP: last dim = it)
        colt = pre.tile([P, NCOLP], F32)
        for jt in range(1, NB):
            for half in range(2):  # 0: A quarters, 1: B quarters
                src = colacc[:, 0:jt, jt, half * 8:(half + 1) * 8]
                src = src.rearrange("p i q -> p q i")
                nc.vector.tensor_reduce(
                    colt[:, (jt - 1) * 16 + half * 8:(jt - 1) * 16 + (half + 1) * 8],
                    src, AX.X, ALU.add)
        nc.scalar.dma_start(out=colp.ap(), in_=colt)

    nc.compile()
    return nc


_NC_CACHE = None


def _get_nc():
    global _NC_CACHE
    if _NC_CACHE is None:
        _NC_CACHE = _build()
    return _NC_CACHE


def _row_index(k):
    """Global row indices owned by core k (i-tiles 8*it + k)."""
    idx = []
    for it_ in range(NIT):
        t = 8 * it_ + k
        idx.append(np.arange(t * P, (t + 1) * P))
    return np.concatenate(idx)


def _make_in_maps(inputs, targets, spectators):
    x = np.ascontiguousarray(np.asarray(inputs, dtype=np.float32)).reshape(N, 1)
    t = np.ascontiguousarray(np.asarray(targets, dtype=np.float32)).reshape(N, 1)
    s = np.ascontiguousarray(np.asarray(spectators, dtype=np.float32)).reshape(N)
    in_maps = []
    for k in range(NCORES):
        idx = _row_index(k)
        in_maps.append({
            "x_full": x,
            "s_full": s,
            "x_strip": np.ascontiguousarray(x[idx]),
            "t_strip": np.ascontiguousarray(t[idx]),
            "s_strip": np.ascontiguousarray(s[idx]),
        })
    return in_maps


def _combine(results):
    """results: list of per-core dicts with partials/rowparts/colparts."""
    g = np.zeros(NOUT, np.float64)
    A = np.zeros(N, np.float64)
    B = np.zeros(N, np.float64)
    for k in range(NCORES):
        g += results[k]["partials"].astype(np.float64)
        rowpart = results[k]["rowparts"].astype(np.float64)  # [128, 16]
        idx = _row_index(k)
        A[idx] += rowpart[:, 0:8].T.reshape(-1)
        B[idx] += rowpart[:, 8:16].T.reshape(-1)
        colpart = results[k]["colparts"].astype(np.float64)  # [128, 7*16]
        cp = colpart.reshape(P, 7, 16)
        # col index (jt-1)*16 + q (A) / + 8 + q (B); j = jt*1024 + q*128 + p
        Ac = cp[:, :, 0:8].transpose(1, 2, 0).reshape(-1)   # [7*8*128] j-ordered
        Bc = cp[:, :, 8:16].transpose(1, 2, 0).reshape(-1)
        A[JT:] += Ac
        B[JT:] += Bc

    cnt, smp, smp2, sms, sms2, bce_sum = g[6], g[7], g[8], g[9], g[10], g[11]
    Sxy = g[5] + 2.0 * g[12]
    sAB = float(A @ B)
    sAA = float(A @ A)
    sBB = float(B @ B)
    Tx = float(A.sum())
    Ty = float(B.sum())

    bce = bce_sum / N
    c = max(cnt, 1.0)
    Sxx = 2.0 * c * smp2 - 2.0 * smp * smp
    Syy = 2.0 * c * sms2 - 2.0 * sms * sms
    Vxy = Sxy - (2.0 / c) * sAB + Tx * Ty / (c * c)
    Vxx = Sxx - (2.0 / c) * sAA + Tx * Tx / (c * c)
    Vyy = Syy - (2.0 / c) * sBB + Ty * Ty / (c * c)
    EPS = 1e-8
    dcov = np.sqrt(max(Vxy / (c * c), EPS))
    dvx = np.sqrt(max(Vxx / (c * c), EPS))
    dvy = np.sqrt(max(Vyy / (c * c), EPS))
    dcor = dcov / (dvx * dvy)
    loss = bce + (dcor if cnt > 0 else 0.0)
    return np.float32(loss)


def kernel(inputs, targets, spectators):
    nc = _get_nc()
    in_maps = _make_in_maps(inputs, targets, spectators)
    res = run_bass_kernel_spmd(nc, in_maps, list(range(NCORES)))
    return _combine(res.results)


if __name__ == "__main__":
    d = np.load("/root/problem/cached_io.npz")
    out = kernel(d["inputs"], d["targets"], d["spectators"])
    exp = float(d["expected"])
    rel = abs(float(out) - exp) / abs(exp)
    print(f"kernel: {float(out):.8f}  expected: {exp:.8f}  rel err: {rel:.3e}")



# revision 11
# speedup vs baseline: 3.4524x; 3.4524x over previous
"""Trainium2 Bass kernel for nn_ConditionalDisCoLoss.

loss = BCEWithLogits(inputs, targets)
     + dCor_masked(sigmoid(inputs), spectators, mask=spectators>=0.5)

Strategy (v3):
  * Compaction: only samples with m=1 (s >= 0.5) contribute to the dCor
    term.  The host gathers the c selected samples and zero-pads to
    C = ceil(c/1024)*1024 (padded rows keep m=0 and drop out exactly).
    For the graded inputs c ~ N/2, so the pair matrix shrinks ~4x.
  * Device computes ONLY the O(C^2) term that needs the pair matrix:
        Sxy = sum_ij m_i m_j |p_i - p_j| |s_i - s_j|
    via W = |D1 * D2'| where D1 = m_i m_j (p_i - p_j) (PE matmul, exact
    masking) and D2' = (s_j - s_i) UNMASKED and UNSIGNED-WRONG on
    purpose: wherever masks matter D1 is already 0, and the final abs
    absorbs the sign.  So D2' never touches PE/PSUM: it is built in
    SBUF bf16 by one DVE tensor_scalar (broadcast s-row minus a
    per-partition scalar) running in the 4x perf mode.
    Per [128 x 1024] tile:
      PE  : D1 via K=2 f32r matmul (m, m*p moving; m*p, -m stationary)
      ACT : a = |D1|  (PSUM f32 -> SBUF bf16)
      DVE : b = s_row - s_i (bf16, 4x), product slice (bf16, 2x),
            W = |a*b| + fused row-sum accumulation (bf16, 4x)
      Pool: the other product slice (SBUF-only; GPSIMD can't touch PSUM)
  * Symmetry: global i-tiles dealt round-robin (core k owns i-tiles
    8*it + k, living in band it), each core computes j-bands jt >= it;
    diag-band tiles count within-band pairs twice across cores, upper
    tiles once (doubled on host): Sxy = Rdiag + 2*Rupper.
  * BCE computed on-device over the (uncompacted) batch, sharded N/8.
  * The O(c log c) 1-D row-sum vectors A_i = sum_j m_j |p_i-p_j| (and B),
    plus the closed-form Sxx/Syy, are exact float64 on the host via
    sorting + prefix sums; they combine with the device scalars into the
    final loss.
"""

import numpy as np
import ml_dtypes
from contextlib import ExitStack

import concourse.bass as bass
import concourse.bacc as bacc
import concourse.tile as tile
from concourse import mybir
from concourse.bass_utils import run_bass_kernel_spmd

N = 8192
NCORES = 8
P = 128
JT = 1024                # j-band width
BSTRIP = N // NCORES     # BCE rows per core

F32 = mybir.dt.float32
BF16 = mybir.dt.bfloat16
F32R = mybir.dt.float32r
ALU = mybir.AluOpType
ACTF = mybir.ActivationFunctionType
AX = mybir.AxisListType

PW_UP = 460              # product columns on Pool (upper tiles); rest on DVE
PW_DIAG = 590            # product columns on Pool (diag tiles)
NOUT = 4                 # out cols: 0 diag_pos, 1 diag_neg, 2 upper_pos, 3 bce
EPS = 1e-8


def _build(C):
    """Build the per-core module for compacted size C (multiple of 1024)."""
    NB = C // JT                 # j-bands == i-tiles per core
    STRIP = C // NCORES          # gathered rows per core (NB * 128)
    FF = C // P                  # full-vector free dim
    PS = 16
    FS = STRIP // PS             # strip tile free dim

    nc = bacc.Bacc("TRN2", target_bir_lowering=False, debug=False,
                   num_devices=NCORES, enable_asserts=False)

    xc_full = nc.dram_tensor("xc_full", [C], F32, kind="ExternalInput")
    sc_full = nc.dram_tensor("sc_full", [C], F32, kind="ExternalInput")
    sc_bf = nc.dram_tensor("sc_bf", [C], BF16, kind="ExternalInput")
    xc_strip = nc.dram_tensor("xc_strip", [STRIP], F32, kind="ExternalInput")
    sc_strip = nc.dram_tensor("sc_strip", [STRIP], F32, kind="ExternalInput")
    xb_strip = nc.dram_tensor("xb_strip", [BSTRIP], F32, kind="ExternalInput")
    tb_strip = nc.dram_tensor("tb_strip", [BSTRIP], F32, kind="ExternalInput")
    out = nc.dram_tensor("cat", [P, NOUT], F32, kind="ExternalOutput")

    with tile.TileContext(nc) as tc, ExitStack() as ctx:
        pre = ctx.enter_context(tc.tile_pool(name="pre", bufs=1))
        uvp = ctx.enter_context(tc.tile_pool(name="uv", bufs=3))
        accp = ctx.enter_context(tc.tile_pool(name="acc", bufs=2))
        psp = ctx.enter_context(tc.tile_pool(name="psp", bufs=4, space="PSUM"))

        # ---------- broadcast s-row (moving side of D2') ----------
        SMrep = pre.tile([P, C], BF16)
        src_row = sc_bf.ap().rearrange("(o c) -> o c", o=1)
        for b in range(NB):
            eng = (nc.sync, nc.scalar)[b % 2]
            eng.dma_start(
                out=SMrep[:, b * JT:(b + 1) * JT],
                in_=src_row[:, b * JT:(b + 1) * JT].partition_broadcast(P))

        # per-partition scalars: s_i for i-tile it lives at sscol[:, it]
        sscol = pre.tile([P, NB], F32)
        nc.sync.dma_start(out=sscol,
                          in_=sc_strip.ap().rearrange("(t p) -> p t", p=P))

        # ---------- preprocessing: full compacted vectors -> moving ops ----------
        xf = pre.tile([P, FF], F32)
        sf = pre.tile([P, FF], F32)
        nc.sync.dma_start(out=xf, in_=xc_full.ap().rearrange("(p f) -> p f", p=P))
        nc.scalar.dma_start(out=sf, in_=sc_full.ap().rearrange("(p f) -> p f", p=P))

        pf = pre.tile([P, FF], F32)
        nc.scalar.activation(pf, xf, ACTF.Sigmoid)
        mf = pre.tile([P, FF], F32)
        nc.vector.tensor_scalar(mf, sf, 0.5, None, ALU.is_ge)
        af = pre.tile([P, FF], F32)
        nc.vector.tensor_tensor(out=af, in0=mf, in1=pf, op=ALU.mult)

        # moving operand: RA rows (m, m*p)
        RA = pre.tile([2, C], F32)
        nc.sync.dma_start(out=RA[0:1, :], in_=mf)
        nc.scalar.dma_start(out=RA[1:2, :], in_=af)

        # ---------- preprocessing: gathered strip -> stationary operand ----------
        xs = pre.tile([PS, FS], F32)
        ss = pre.tile([PS, FS], F32)
        nc.sync.dma_start(out=xs, in_=xc_strip.ap().rearrange("(p f) -> p f", p=PS))
        nc.scalar.dma_start(out=ss, in_=sc_strip.ap().rearrange("(p f) -> p f", p=PS))

        ps_ = pre.tile([PS, FS], F32)
        nc.scalar.activation(ps_, xs, ACTF.Sigmoid)
        ms = pre.tile([PS, FS], F32)
        nc.vector.tensor_scalar(ms, ss, 0.5, None, ALU.is_ge)
        negm = pre.tile([PS, FS], F32)
        nc.vector.tensor_scalar(negm, ms, -1.0, None, ALU.mult)
        bs = pre.tile([PS, FS], F32)
        nc.vector.tensor_tensor(out=bs, in0=ms, in1=ps_, op=ALU.mult)

        # stationary operand: LA rows (m*p, -m)
        LA = pre.tile([2, STRIP], F32)
        nc.sync.dma_start(out=LA[0:1, :], in_=bs)
        nc.scalar.dma_start(out=LA[1:2, :], in_=negm)

        # ---------- output accumulators ----------
        cat = pre.tile([P, NOUT], F32)
        nc.vector.memset(cat, 0.0)
        RDp = pre.tile([P, NB], F32)
        RDn = pre.tile([P, NB], F32)
        RUp = pre.tile([P, NB], F32)
        nc.vector.memset(RUp, 0.0)

        # ---------- BCE over the uncompacted batch strip ----------
        BF = BSTRIP // PS
        xbt = pre.tile([PS, BF], F32)
        tbt = pre.tile([PS, BF], F32)
        nc.sync.dma_start(out=xbt, in_=xb_strip.ap().rearrange("(p f) -> p f", p=PS))
        nc.scalar.dma_start(out=tbt, in_=tb_strip.ap().rearrange("(p f) -> p f", p=PS))
        rx = pre.tile([PS, BF], F32)
        nc.vector.tensor_scalar(rx, xbt, 0.0, None, ALU.max)
        xt = pre.tile([PS, BF], F32)
        nc.vector.tensor_tensor(out=xt, in0=xbt, in1=tbt, op=ALU.mult)
        axx = pre.tile([PS, BF], F32)
        nc.scalar.activation(axx, xbt, ACTF.Abs)
        enx = pre.tile([PS, BF], F32)
        nc.scalar.activation(enx, axx, ACTF.Exp, scale=-1.0)
        sp = pre.tile([PS, BF], F32)
        nc.scalar.activation(sp, enx, ACTF.Ln, bias=1.0)
        t1 = pre.tile([PS, BF], F32)
        nc.vector.tensor_tensor(out=t1, in0=rx, in1=xt, op=ALU.subtract)
        t2 = pre.tile([PS, BF], F32)
        nc.vector.scalar_tensor_tensor(out=t2, in0=t1, scalar=0.0, in1=sp,
                                       op0=ALU.add, op1=ALU.add,
                                       accum_out=cat[0:PS, 3:4])

        # ---------- main pass: tiles (it, jt) with jt >= it ----------
        # The compacted array is sorted by s, so upper tiles (jt > it) have
        # s_j - s_i >= 0 everywhere: a single positive accumulation.  Diag
        # tiles have mixed signs: accumulate max(P,0) and min(P,0) separately
        # (host combines pos - neg = sum |P|).
        for it in range(NB):
            njt = NB - it
            Rp = accp.tile([P, NB + 1], F32, tag="Rp")
            lA = LA[:, it * P:(it + 1) * P]
            for jj in range(njt):
                jt = it + jj
                psA = psp.tile([P, JT], F32, tag="ps")
                for h in range(JT // 512):
                    j0 = jt * JT + h * 512
                    nc.tensor.matmul(psA[:, h * 512:(h + 1) * 512],
                                     lhsT=lA.bitcast(F32R),
                                     rhs=RA[:, j0:j0 + 512].bitcast(F32R),
                                     start=True, stop=True)
                a_s = uvp.tile([P, JT], BF16, tag="a")
                nc.scalar.activation(a_s, psA, ACTF.Abs)
                b_s = uvp.tile([P, JT], BF16, tag="b")
                nc.vector.tensor_scalar(b_s, SMrep[:, jt * JT:(jt + 1) * JT],
                                        sscol[:, it:it + 1], None, ALU.subtract)
                Pp = uvp.tile([P, JT], BF16, tag="Pp")
                pw = PW_DIAG if jj == 0 else PW_UP
                nc.gpsimd.tensor_tensor(out=Pp[:, 0:pw],
                                        in0=a_s[:, 0:pw],
                                        in1=b_s[:, 0:pw], op=ALU.mult)
                nc.vector.tensor_tensor(out=Pp[:, pw:JT],
                                        in0=a_s[:, pw:JT],
                                        in1=b_s[:, pw:JT], op=ALU.mult)
                Wj = uvp.tile([P, JT], BF16, tag="Wj")
                if jj == 0:
                    nc.vector.tensor_scalar(Wj, Pp, 0.0, None, ALU.max,
                                            ALU.add, accum_out=Rp[:, 0:1])
                    Wn = uvp.tile([P, JT], BF16, tag="Wn")
                    nc.vector.tensor_scalar(Wn, Pp, 0.0, None, ALU.min,
                                            ALU.add, accum_out=Rp[:, 1:2])
                else:
                    nc.vector.tensor_scalar(Wj, Pp, 0.0, None, ALU.max,
                                            ALU.add, accum_out=Rp[:, jj + 1:jj + 2])
            nc.vector.tensor_copy(RDp[:, it:it + 1], Rp[:, 0:1])
            nc.vector.tensor_copy(RDn[:, it:it + 1], Rp[:, 1:2])
            if njt > 1:
                nc.vector.tensor_reduce(RUp[:, it:it + 1], Rp[:, 2:njt + 1],
                                        AX.X, ALU.add)

        # ---------- outputs ----------
        nc.vector.tensor_reduce(cat[:, 0:1], RDp, AX.X, ALU.add)
        nc.vector.tensor_reduce(cat[:, 1:2], RDn, AX.X, ALU.add)
        nc.vector.tensor_reduce(cat[:, 2:3], RUp, AX.X, ALU.add)
        nc.sync.dma_start(out=out.ap(), in_=cat)

    nc.compile()
    return nc


_NC_CACHE = {}


def _get_nc(C):
    if C not in _NC_CACHE:
        _NC_CACHE[C] = _build(C)
    return _NC_CACHE[C]


def _row_index(k, C):
    """Strip rows for core k: i-tiles 8*it + k of the compacted array."""
    nit = C // JT
    return np.concatenate([np.arange((8 * it + k) * P, (8 * it + k + 1) * P)
                           for it in range(nit)])


def _make_in_maps(x, t, s, C):
    m = s >= 0.5
    idx = np.flatnonzero(m)
    idx = idx[np.argsort(s[idx], kind="stable")]   # sorted by s ascending
    xc = np.zeros(C, np.float32)
    sc = np.zeros(C, np.float32)
    npad = C - idx.size
    xc[npad:] = x[idx]                             # pad (s=0, m=0) sorts first
    sc[npad:] = s[idx]
    sc_bf = sc.astype(ml_dtypes.bfloat16)
    in_maps = []
    for k in range(NCORES):
        ridx = _row_index(k, C)
        in_maps.append({
            "xc_full": xc,
            "sc_full": sc,
            "sc_bf": sc_bf,
            "xc_strip": np.ascontiguousarray(xc[ridx]),
            "sc_strip": np.ascontiguousarray(sc[ridx]),
            "xb_strip": np.ascontiguousarray(x[k * BSTRIP:(k + 1) * BSTRIP]),
            "tb_strip": np.ascontiguousarray(t[k * BSTRIP:(k + 1) * BSTRIP]),
        })
    return in_maps


def _combine(results, x, s):
    g = np.zeros(NOUT, np.float64)
    for k in range(NCORES):
        g += results[k]["cat"].astype(np.float64).sum(axis=0)
    Sxy = (g[0] - g[1]) + 2.0 * g[2]
    bce = g[3] / N

    x64 = x.astype(np.float64)
    s64 = s.astype(np.float64)
    m = s >= 0.5
    c = int(m.sum())
    if c == 0:
        return np.float32(bce)
    pv = 1.0 / (1.0 + np.exp(-x64[m]))
    sv = s64[m]

    def rowsums(v):
        order = np.argsort(v, kind="stable")
        vs = v[order]
        pre = np.cumsum(vs)
        r = np.arange(c)
        tot = pre[-1]
        a_sorted = vs * (r + 1) - pre + (tot - pre) - vs * (c - 1 - r)
        outv = np.empty(c)
        outv[order] = a_sorted
        return outv

    A = rowsums(pv)
    B = rowsums(sv)
    sAB = float(A @ B)
    sAA = float(A @ A)
    sBB = float(B @ B)
    Tx = float(A.sum())
    Ty = float(B.sum())
    smp, smp2 = pv.sum(), (pv * pv).sum()
    sms, sms2 = sv.sum(), (sv * sv).sum()

    cc = float(c)
    Sxx = 2.0 * cc * smp2 - 2.0 * smp * smp
    Syy = 2.0 * cc * sms2 - 2.0 * sms * sms
    Vxy = Sxy - (2.0 / cc) * sAB + Tx * Ty / (cc * cc)
    Vxx = Sxx - (2.0 / cc) * sAA + Tx * Tx / (cc * cc)
    Vyy = Syy - (2.0 / cc) * sBB + Ty * Ty / (cc * cc)
    dcov = np.sqrt(max(Vxy / (cc * cc), EPS))
    dvx = np.sqrt(max(Vxx / (cc * cc), EPS))
    dvy = np.sqrt(max(Vyy / (cc * cc), EPS))
    return np.float32(bce + dcov / (dvx * dvy))


def kernel(inputs, targets, spectators):
    x = np.ascontiguousarray(np.asarray(inputs, dtype=np.float32)).reshape(N)
    t = np.ascontiguousarray(np.asarray(targets, dtype=np.float32)).reshape(N)
    s = np.ascontiguousarray(np.asarray(spectators, dtype=np.float32)).reshape(N)
    c = int((s >= 0.5).sum())
    C = max(JT, ((c + JT - 1) // JT) * JT)
    nc = _get_nc(C)
    in_maps = _make_in_maps(x, t, s, C)
    res = run_bass_kernel_spmd(nc, in_maps, list(range(NCORES)))
    return _combine(res.results, x, s)


if __name__ == "__main__":
    d = np.load("/root/problem/cached_io.npz")
    out = kernel(d["inputs"], d["targets"], d["spectators"])
    exp = float(d["expected"])
    rel = abs(float(out) - exp) / abs(exp)
    print(f"kernel: {float(out):.8f}  expected: {exp:.8f}  rel err: {rel:.3e}")
